# revision 116
# baseline (speedup 1.0000x reference)
"""Trainium2 Bass kernel: 16-head attention block (B=2, S=2048, H=1024).

Sharding: 8 cores = 2-way data parallel (batch) x 4-way tensor parallel
(head groups of 4 heads / 256 dims). Each core computes, for its batch
and head group:
    Q^T, K^T (= W @ x^T, [dims, seq] layout; Wq/bq pre-scaled by 1/8 on
    host so no score scaling is needed on device), V ([seq, dims]),
    S^T = K Q^T per head (key positions on partitions),
    P^T = exp(S^T + mask),
    ctx'^T = [V | 1]^T P^T    (ones column folded in -> row 64 = softmax
                               denominator),
    ctx^T normalized, then partial output O_g = ctx^T.T @ Wo[:,hs]^T.
Host sums the 4 partial outputs per batch and adds bo.

Projections run as fp8-e4m3 DoubleRow matmuls (hi/lo split pairs, 3
significant pairings, weights host-scaled by 64); attention matmuls run
as float32r (full-rate fp32 mode on the PE array). Emission order
interleaves pair-1 projections under pair-0's ACT-bound attention, and
the output projection under pair-1's attention, so the PE fills the
softmax (scalar-engine) shadow; each chunk's normalization tail stays
short via the wo_x re-staged weight trick and ACT-assisted copies.
"""

import contextlib

import numpy as np

import concourse.bass as bass
import concourse.mybir as mybir
import concourse.tile as tile
from concourse import bacc
from concourse.bass_utils import run_bass_kernel_spmd

B, S, H = 2, 2048, 1024
NUM_HEADS, HEAD_DIM = 16, 64
N_CORES = 8
GROUPS = 4                  # head-parallel groups per batch
HD = H // GROUPS            # 256 head-dims per core (4 heads)
P = 128
KT_H = H // P               # 8 k-tiles over hidden dim
KT_S = S // P               # 16 k-tiles over sequence (key positions)
NCH = 4                     # q chunks
CHUNK = S // NCH            # 512
F32 = mybir.dt.float32
F32R = mybir.dt.float32r
BF16 = mybir.dt.bfloat16
E4M3 = mybir.dt.float8e4
DR = mybir.MatmulPerfMode.DoubleRow
EXP = mybir.ActivationFunctionType.Exp
# Q/K/V weights and x are sent as hi/lo e4m3 pairs (W pre-scaled by 64 into
# e4m3's normal range); projections run as fp8 DoubleRow matmuls (2 k-tiles
# per instruction at 0.5 cycles/row) over the 3 significant hi/lo pairings.
# The 64*64 scale on q*k is undone in the exp (scale 2^-15 includes the /8
# softmax scaling); the 64 on v is undone in the recip broadcast (1/64).
EXP_SCALE = float(2.0 ** -15)
DKT = KT_H // 2            # 4 double-k-tiles per 1024-deep contraction

_PROGRAM_CACHE = {}
# 0: plain last chunk; 1: 256-query sub-chunks, normal norm/oproj;
# 2: sub-chunks + psum-recip/tmp_o norm + wo_x output projection;
# 3: full-width last chunk with psum-recip/tmp_o norm + wo_x oproj
TAIL_MODE = 3


def _emit(tc, nc, dram, masked, with_bias):
    mm = nc.tensor.matmul
    (x_hi_d, x_lo_d, wq_hl_d, wk_hl_d, wv_hl_d,
     wo_d, bq_d, bk_d, bv_d, am_d, o_d, _unused) = dram

    stack = contextlib.ExitStack()
    with stack:
        const = stack.enter_context(tc.tile_pool(name="const", bufs=1))
        big = stack.enter_context(tc.tile_pool(name="big", bufs=1))

        onesf = const.tile([P, 64], F32)
        nc.any.memset(onesf[:], 1.0)
        inv64f = const.tile([P, 64], F32)
        nc.any.memset(inv64f[:], 1.0 / 64.0)
        ones64 = const.tile([P, 64], F32R)   # lane-64 row used as K=1 lhsT
        nc.vector.tensor_copy(ones64[:], inv64f[:])
        # warm the ACT exp table before it is first needed
        trash = const.tile([1, 16], F32)
        nc.scalar.activation(trash[:], onesf[0:1, 0:16], EXP)
        if masked:
            amask_sb = const.tile([P, KT_S], F32)
            nc.sync.dma_start(out=amask_sb[:], in_=am_d[:])
        if with_bias:
            ones_sb = const.tile([1, 512], F32R)
            for i in range(8):
                nc.vector.tensor_copy(ones_sb[0:1, i * 64:(i + 1) * 64],
                                      onesf[0:1, :])
            bq_sb = const.tile([1, HD], F32R)
            nc.sync.dma_start(out=bq_sb[:], in_=bq_d[:])
            bk_sb = const.tile([1, HD], F32R)
            nc.sync.dma_start(out=bk_sb[:], in_=bk_d[:])
            bv_sb = const.tile([1, HD], F32R)
            nc.sync.dma_start(out=bv_sb[:], in_=bv_d[:])
        wo_sb = const.tile([P, HD // P, H], BF16)
        # wo rows 192-255 staged again at partitions 0-63: lets the last
        # sub-chunk's odd-head output projection read straight from the
        # tmp_o normalization buffer instead of waiting on the SBUF-SBUF
        # DMA into ctxT partitions 64-127 (a ~2.3us latency chain).
        wo_x = const.tile([HEAD_DIM, H], BF16)

        # persistent activations
        qT_sb = big.tile([P, 2, S], F32R)    # [dim-in-pair, pair, seq]
        kT_sb = big.tile([P, 2, S], F32R)
        v_sb = big.tile([P, KT_S, GROUPS, HEAD_DIM + 1], F32R)  # [seq, kt, head, d+1]
        ctxT_sb = big.tile([P, 2, S], BF16)

        # ones column of V' (the rowsum trick)
        nc.vector.tensor_copy(v_sb[:, :, :, HEAD_DIM:HEAD_DIM + 1], onesf[:, 0:KT_S * GROUPS])

        # ---------- input DMAs (wk/xT first so compute starts early) ----------
        kq_pool = tc.alloc_tile_pool(name="kq_pool", bufs=1, side="right")
        wv_stack = contextlib.ExitStack()
        wv_pool = wv_stack.enter_context(tc.tile_pool(name="wv_pool", bufs=1, side="right"))
        wk_t = kq_pool.tile([P, KT_H, 2, HD], E4M3)
        x_hi = kq_pool.tile([P, KT_H, S], E4M3)
        x_lo = kq_pool.tile([P, KT_H, S], E4M3)
        wq_t = kq_pool.tile([P, KT_H, 2, HD], E4M3)
        wv_t = wv_pool.tile([P, KT_H, 2, HD], E4M3)
        # hi/lo interleave per dram row keeps the DMA's contiguous runs at
        # 512B (256B runs pay a 2x descriptor latency penalty)
        wk_hi, wk_lo = wk_t[:, :, 0, :], wk_t[:, :, 1, :]
        wq_hi, wq_lo = wq_t[:, :, 0, :], wq_t[:, :, 1, :]
        wv_hi, wv_lo = wv_t[:, :, 0, :], wv_t[:, :, 1, :]
        # One dma_start costs ~625ns of serialized HWDGE issue time and
        # transfers serialize on the DMA engines, so move whole tensors per
        # DMA — but peel kt-0 of wk/wq/x so the first K/Q matmul can start
        # ~4us in instead of waiting for the full-tensor transfers.
        # Transfers serialize on the DMA engines (~332 GB/s) and the inbound
        # set needed in the first ~18us totals ~5.5MB, so the issue order
        # must exactly track first-use order: kt0 of wk/wq/x (first matmul
        # ~4.5us in), the rest of wk/wq, x-cc0 by kt interleaved with the
        # later x chunks, wv just before the V tiles, wo/wo_x last.
        def x_cc(dst, src, cc):
            nc.sync.dma_start(
                out=dst[:, :, cc * 512:(cc + 1) * 512],
                in_=src[:, cc * 512:(cc + 1) * 512]
                .rearrange("(t p) c -> p t c", p=P))

        # first double-k-tile (kt 0-1) of the hi tensors unlocks the first
        # hi*hi projection matmul ~4.5us in; lo tensors follow
        nc.sync.dma_start(out=wk_t[:, 0:2, :, :],
                          in_=wk_hl_d[0:2 * P].rearrange("(t p) u c -> p t u c", p=P))
        nc.sync.dma_start(out=wq_t[:, 0:2, :, :],
                          in_=wq_hl_d[0:2 * P].rearrange("(t p) u c -> p t u c", p=P))
        nc.sync.dma_start(out=x_hi[:, 0:2, 0:512],
                          in_=x_hi_d[0:2 * P, 0:512].rearrange("(t p) c -> p t c", p=P))
        nc.sync.dma_start(out=wk_t[:, 2:KT_H, :, :],
                          in_=wk_hl_d[2 * P:].rearrange("(t p) u c -> p t u c", p=P))
        nc.sync.dma_start(out=wq_t[:, 2:KT_H, :, :],
                          in_=wq_hl_d[2 * P:].rearrange("(t p) u c -> p t u c", p=P))
        for dk in range(1, DKT):
            nc.sync.dma_start(
                out=x_hi[:, 2 * dk:2 * dk + 2, 0:512],
                in_=x_hi_d[2 * dk * P:(2 * dk + 2) * P, 0:512]
                .rearrange("(t p) c -> p t c", p=P))
        nc.sync.dma_start(out=x_lo[:, :, 0:512],
                          in_=x_lo_d[:, 0:512].rearrange("(t p) c -> p t c", p=P))
        x_cc(x_hi, x_hi_d, 1)
        x_cc(x_lo, x_lo_d, 1)
        x_cc(x_hi, x_hi_d, 2)
        x_cc(x_lo, x_lo_d, 2)
        nc.sync.dma_start(out=wv_t[:], in_=wv_hl_d.rearrange("(t p) u c -> p t u c", p=P))
        x_cc(x_hi, x_hi_d, 3)
        x_cc(x_lo, x_lo_d, 3)
        nc.sync.dma_start(out=wo_sb[:], in_=wo_d.rearrange("(t p) c -> p t c", p=P))
        nc.sync.dma_start(out=wo_x[:], in_=wo_d[3 * HEAD_DIM:4 * HEAD_DIM, :])

        # all projection psum traffic lives in two persistent banks (tags
        # ps_k / ps_q, alternating for double-buffering) so the attention
        # pool can hold its six banks for the whole kernel with no
        # pool-boundary barrier between projections and attention
        def dr_gen(ps, ahi, alo, bhi, blo, ac, bc_, bias_mm=None):
            """fp8 DoubleRow accumulation over the 3 significant hi/lo
            pairings x 4 double-k-tiles; yields once per PE matmul. Pairing
            order (hi*hi, lo*hi, hi*lo) matches DMA arrival order."""
            a0, a1 = ac
            b0, b1 = bc_
            total = 3 * DKT + (1 if bias_mm is not None else 0)
            n = 0
            for at, bt in ((ahi, bhi), (alo, bhi), (ahi, blo)):
                for j in range(DKT):
                    mm(ps[:],
                       at[:, 2 * j:2 * j + 2, a0:a1],
                       bt[:, 2 * j:2 * j + 2, b0:b1],
                       start=(n == 0), stop=(n == total - 1), perf_mode=DR)
                    n += 1
                    yield
            if bias_mm is not None:
                bias_mm()
                yield

        def qk_proj_gen(ps_pool, whi, wlo, b_sb, dst, p):
            """Yields once per PE matmul so the caller can interleave these
            under the ACT-bound attention kt loop."""
            for c4 in range(4):
                ps_qk = ps_pool.tile([P, 512], F32, bufs=1, name="ps_qk",
                                     tag="ps_k" if c4 % 2 == 0 else "ps_q")
                bias_mm = None
                if with_bias:
                    def bias_mm(ps_qk=ps_qk):
                        mm(ps_qk[:],
                           b_sb[:, p * P:(p + 1) * P],
                           ones_sb[:, 0:512],
                           start=False, stop=True)
                yield from dr_gen(ps_qk, whi, wlo, x_hi, x_lo,
                                  (p * P, (p + 1) * P),
                                  (c4 * 512, (c4 + 1) * 512), bias_mm)
                nc.vector.tensor_copy(dst[:, p, c4 * 512:(c4 + 1) * 512], ps_qk[:])

        def oproj_gen(ps_pool, opool, ms):
            """Yields once per PE matmul; each output half DMAs out as soon
            as its psum copy lands so the final-tile chain stays short.
            PSUM rides the ps_k/ps_q tag pair (free once KQ-1 is done)."""
            for m in ms:
                o_sb = opool.tile([P, H], BF16, tag="o_sb", bufs=3)
                for n2 in range(2):
                    ps_o = ps_pool.tile([P, 512], F32, name="ps_o", bufs=1,
                                        tag="ps_k" if n2 == 0 else "ps_q")
                    for kt2 in range(HD // P):
                        mm(ps_o[:],
                           ctxT_sb[:, kt2, m * P:(m + 1) * P],
                           wo_sb[:, kt2, n2 * 512:(n2 + 1) * 512],
                           start=(kt2 == 0), stop=(kt2 == HD // P - 1))
                        yield
                    nc.vector.tensor_copy(o_sb[:, n2 * 512:(n2 + 1) * 512], ps_o[:])
                    nc.sync.dma_start(
                        out=o_d[m * P:(m + 1) * P, n2 * 512:(n2 + 1) * 512],
                        in_=o_sb[:, n2 * 512:(n2 + 1) * 512])

        def v_tile(m, pool):
            ps_v = pool.tile([P, HD], F32, bufs=1, name="ps_v",
                             tag="ps_k" if m % 2 == 0 else "ps_q")
            bias_mm = None
            if with_bias:
                def bias_mm():
                    mm(ps_v[:],
                       ones_sb[:, 0:P],
                       bv_sb[:],
                       start=False, stop=True)
            for _ in dr_gen(ps_v, x_hi, x_lo, wv_hi, wv_lo,
                            (m * P, (m + 1) * P), (0, HD), bias_mm):
                pass
            nc.vector.tensor_copy(v_sb[:, m, :, 0:HEAD_DIM], ps_v[:])

        def v_tile_gen(ms, pool):
            for m in ms:
                v_tile(m, pool)
                yield

        # ---------- pools: attention psum first (banks 0-5, alive for the
        # whole kernel), projections in the remaining two banks ----------
        attn_stack = contextlib.ExitStack()
        a_ps = attn_stack.enter_context(
            tc.tile_pool(name="attn_psum", bufs=1, space="PSUM"))
        ptp = attn_stack.enter_context(
            tc.tile_pool(name="pt_pool", bufs=4 if not (masked or with_bias) else 2))
        npool = attn_stack.enter_context(tc.tile_pool(name="norm_pool", bufs=2))
        psA = tc.alloc_tile_pool(name="proj_psum", bufs=1, space="PSUM")

        # pair-0 K/Q projections, seq-chunk-outer so each inbound x
        # column-chunk is consumed as soon as it lands; K and Q steps
        # interleave so each unlocks on the same DMA arrivals
        def p0_one(cc, whi, wlo, b_sb, dst, tag):
            """One pair-0 projection chunk (K or Q) for seq-chunk cc."""
            ps0 = psA.tile([P, 512], F32, tag=tag, bufs=1, name="ps0")
            bias_mm = None
            if with_bias:
                def bias_mm():
                    mm(ps0[:], b_sb[:, 0:P], ones_sb[:, 0:512],
                       start=False, stop=True)
            cols = (cc * 512, (cc + 1) * 512)
            for _ in dr_gen(ps0, whi, wlo, x_hi, x_lo, (0, P), cols, bias_mm):
                pass
            nc.vector.tensor_copy(dst[:, 0, cc * 512:(cc + 1) * 512], ps0[:])

        def kq0_cc(cc):
            ps_k = psA.tile([P, 512], F32, tag="ps_k", bufs=1, name="ps_k")
            ps_q = psA.tile([P, 512], F32, tag="ps_q", bufs=1, name="ps_q")
            bias_k = bias_q = None
            if with_bias:
                def bias_k(ps_k=ps_k):
                    mm(ps_k[:], bk_sb[:, 0:P], ones_sb[:, 0:512],
                       start=False, stop=True)

                def bias_q(ps_q=ps_q):
                    mm(ps_q[:], bq_sb[:, 0:P], ones_sb[:, 0:512],
                       start=False, stop=True)
            cols = (cc * 512, (cc + 1) * 512)
            for _ in zip(dr_gen(ps_k, wk_hi, wk_lo, x_hi, x_lo, (0, P), cols, bias_k),
                         dr_gen(ps_q, wq_hi, wq_lo, x_hi, x_lo, (0, P), cols, bias_q)):
                pass
            nc.vector.tensor_copy(kT_sb[:, 0, cc * 512:(cc + 1) * 512], ps_k[:])
            nc.vector.tensor_copy(qT_sb[:, 0, cc * 512:(cc + 1) * 512], ps_q[:])

        def q0_burst(cc):
            p0_one(cc, wq_hi, wq_lo, bq_sb if with_bias else None, qT_sb, "ps_q")

        kq0_cc(0)
        kq0_cc(1)
        kq0_cc(2)
        # V tiles 0,1 pre-rolled; KQ-cc3 waits on the LAST x chunk
        # (~19.7us), but chunk 0's scores only read kT-cc3 at kt12 — so it
        # rides as a filler burst at kt7 and attention starts ~3us earlier,
        # right after KQ-cc2. V tiles 2-8 lead their consumers by 2 kts,
        # 9-15 by 1 kt.
        v_tile(0, psA)
        v_tile(1, psA)

        def c0_fill():
            for m in range(2, 9):
                v_tile(m, psA)
                yield
            kq0_cc(3)
            yield
            for m in range(9, KT_S):
                v_tile(m, psA)
                yield

        vfill = c0_fill()

        def attn(p, qlo, qw, filler, last=False, fill_n=1, fill2=0,
                 pre=None, defer_norm=False):
            return _attn_one_chunk(tc, nc, a_ps, ptp, npool, p, qlo, qw,
                                   masked, amask_sb if masked else None,
                                   kT_sb, qT_sb, v_sb, ctxT_sb, ones64,
                                   filler=filler, last=last, fill_n=fill_n,
                                   fill2=fill2, pre=pre,
                                   defer_norm=defer_norm,
                                   bc_pool=psA if defer_norm else None)

        def oproj_last(ps_pool, opool, ms, tmp_o, qlo):
            """Output projection for the final sub-chunk: the odd-head half
            of pair 1 is read from tmp_o (partitions 0-63) against wo_x, so
            no SBUF-SBUF DMA sits on the critical path. Staging copies
            split across DVE and the now-idle ACT engine so neither
            serializes the whole tail."""
            for m in ms:
                o_sb = opool.tile([P, H], BF16, tag="o_sb", bufs=3)
                for n2 in range(2):
                    ps_o = ps_pool.tile([P, 512], F32, name="ps_o", bufs=1,
                                        tag="ps_k" if n2 == 0 else "ps_q")
                    mm(ps_o[:],
                       ctxT_sb[:, 0, m * P:(m + 1) * P],
                       wo_sb[:, 0, n2 * 512:(n2 + 1) * 512],
                       start=True, stop=False)
                    mm(ps_o[:],
                       ctxT_sb[0:64, 1, m * P:(m + 1) * P],
                       wo_sb[0:64, 1, n2 * 512:(n2 + 1) * 512],
                       start=False, stop=False)
                    mm(ps_o[:],
                       tmp_o[:, m * P - qlo:(m + 1) * P - qlo],
                       wo_x[:, n2 * 512:(n2 + 1) * 512],
                       start=False, stop=True)
                    dst = o_sb[:, n2 * 512:(n2 + 1) * 512]
                    if n2 == 0:
                        nc.vector.tensor_copy(dst, ps_o[:])
                    else:
                        nc.scalar.copy(dst, ps_o[:])
                    nc.sync.dma_start(
                        out=o_d[m * P:(m + 1) * P, n2 * 512:(n2 + 1) * 512],
                        in_=dst)

        # Each chunk's normalization is deferred into the next chunk's first
        # kt iteration: its bc broadcast waits ~1.5us on the DVE reciprocal
        # chain, and emitting it before the next chunk's scores starved the
        # ACT exp stream (the critical path) at every chunk boundary.
        attn(0, 0, CHUNK, vfill)
        for _ in vfill:
            pass

        # The attention inner loop is ACT-bound (exp 1038ns/kt vs 852ns/kt
        # of PE work), so projection matmuls interleave into every kt
        # iteration to fill the ~190ns PE bubble: the 96 KQ-1 DoubleRow
        # matmuls cover pair-0 chunks 1-3 and pair-1 chunk 0; output-
        # projection matmuls (chunk c-1's) cover the rest of pair-1.
        import itertools
        kq1 = itertools.chain(
            qk_proj_gen(psA, wk_hi, wk_lo, bk_sb if with_bias else None, kT_sb, 1),
            qk_proj_gen(psA, wq_hi, wq_lo, bq_sb if with_bias else None, qT_sb, 1))

        # pair-0 chunks must consume at least the 48 K plus 12 Q(cols
        # 0-511) KQ-1 matmuls before pair-1 chunk 0's scores read them:
        # 2 fillers/kt for kt<8 gives 24/chunk (72 total).
        for c in range(1, NCH):
            attn(0, c * CHUNK, CHUNK, kq1, fill2=8)
        wv_stack.close()

        opool = attn_stack.enter_context(tc.tile_pool(name="o_pool", bufs=1))
        attn(1, 0, CHUNK, kq1, fill2=8)
        for _ in kq1:
            pass
        kq_pool.release()

        ofill = oproj_gen(psA, opool, [0, 1, 2, 3])
        attn(1, CHUNK, CHUNK, ofill)
        for _ in ofill:
            pass
        ofill = oproj_gen(psA, opool, [4, 5, 6, 7])
        attn(1, 2 * CHUNK, CHUNK, ofill)
        for _ in ofill:
            pass
        if TAIL_MODE == 2:
            # the last chunk runs as two 256-query sub-chunks so the final
            # norm + output-projection tail is half as long
            ofill = oproj_gen(psA, opool, [8, 9, 10, 11])
            attn(1, 3 * CHUNK, CHUNK // 2, ofill)
            for _ in ofill:
                pass
            ofill = oproj_gen(psA, opool, [12, 13])
            qlo_b = 3 * CHUNK + CHUNK // 2
            tmp_b = attn(1, qlo_b, CHUNK // 2, ofill, last=True)
            for _ in ofill:
                pass
            oproj_last(psA, opool, [14, 15], tmp_b, qlo_b)
        elif TAIL_MODE == 3:
            ofill = oproj_gen(psA, opool, [8, 9, 10, 11])
            tmp_b = attn(1, 3 * CHUNK, CHUNK, ofill, last=True)
            for _ in ofill:
                pass
            oproj_last(psA, opool, [12, 13, 14, 15], tmp_b, 3 * CHUNK)
        elif TAIL_MODE == 1:
            ofill = oproj_gen(psA, opool, [8, 9, 10, 11])
            attn(1, 3 * CHUNK, CHUNK // 2, ofill)
            for _ in ofill:
                pass
            ofill = oproj_gen(psA, opool, [12, 13])
            attn(1, 3 * CHUNK + CHUNK // 2, CHUNK // 2, ofill)
            for _ in ofill:
                pass
            for _ in oproj_gen(psA, opool, [14, 15]):
                pass
        else:
            ofill = oproj_gen(psA, opool, [8, 9, 10, 11])
            attn(1, 3 * CHUNK, CHUNK, ofill)
            for _ in ofill:
                pass
            for _ in oproj_gen(psA, opool, [12, 13, 14, 15]):
                pass
        psA.release()
        attn_stack.close()


def _attn_one_chunk(tc, nc, psum, ptp, npool, p, qlo, qw, masked, amask_sb,
                    kT_sb, qT_sb, v_sb, ctxT_sb, ones64, filler=None,
                    last=False, fill_n=1, fill2=0, pre=None,
                    defer_norm=False, bc_pool=None):
    mm = nc.tensor.matmul
    # tiles are always allocated at full chunk width and sliced to qw:
    # shape-varying tiles under one psum tag break on real hardware
    ctx_e = psum.tile([HEAD_DIM + 1, CHUNK], F32, tag="ctx_e", bufs=1,
                      name="ctx_e")[:, 0:qw]
    ctx_o = psum.tile([HEAD_DIM + 1, CHUNK], F32, tag="ctx_o", bufs=1,
                      name="ctx_o")[:, 0:qw]
    for kt in range(KT_S):
        s_pair = psum.tile([P, 2 * CHUNK], F32, tag="s_pair", bufs=2,
                           name="s_pair")[:, 0:2 * qw]
        for hl in range(2):
            mm(s_pair[:, hl * qw:(hl + 1) * qw],
               kT_sb[hl * 64:(hl + 1) * 64, p, kt * P:(kt + 1) * P],
               qT_sb[hl * 64:(hl + 1) * 64, p, qlo:qlo + qw],
               start=True, stop=True)
        pt = ptp.tile([P, 2 * CHUNK], F32R, tag="pt", name="pt")[:, 0:2 * qw]
        if masked:
            nc.scalar.activation(pt[:], s_pair[:], EXP,
                                 bias=amask_sb[:, kt:kt + 1], scale=EXP_SCALE)
        else:
            nc.scalar.activation(pt[:], s_pair[:], EXP, scale=EXP_SCALE)
        for hl in range(2):
            ctx = ctx_e if hl == 0 else ctx_o
            mm(ctx[:],
               v_sb[:, kt, 2 * p + hl, :],
               pt[:, hl * qw:(hl + 1) * qw],
               start=(kt == 0), stop=(kt == KT_S - 1))
        if kt == 0 and pre is not None:
            # previous chunk's deferred normalization: emitting it after
            # this chunk's first scores keeps the ACT exp stream fed across
            # the chunk boundary (the bc matmuls wait ~1.5us on the DVE
            # reciprocal chain and would otherwise stall the PE's in-order
            # wait window before the first scores issue)
            pre()
        if filler is not None:
            n_fill = fill_n + (1 if kt < fill2 else 0)
            for _ in range(n_fill):
                next(filler, None)
    recip_sb = npool.tile([HEAD_DIM + 1, 2, CHUNK], F32R, tag="recip",
                          bufs=2, name="recip_sb")[:, :, 0:qw]
    if last:
        # final sub-chunk: reciprocals read the denominator row straight
        # from psum so the bc broadcast starts early; ctx rows 0-63 still
        # stage through ctxu (the DVE cannot read two psum operands in one
        # tensor_tensor) — on the ACT engine, which is idle once its last
        # exp retires, so the copies run parallel to the reciprocals. The
        # odd half stays in tmp_o: the caller's output projection reads it
        # there (via wo_x), so no SBUF-SBUF DMA sits on the critical path.
        with nc.allow_low_precision(reason="softmax denominators are O(1e3); fp32r's 11-bit mantissa is plenty"):
            nc.vector.reciprocal(recip_sb[64:65, 0, :], ctx_e[64:65, :])
            nc.vector.reciprocal(recip_sb[64:65, 1, :], ctx_o[64:65, :])
        ctxu = npool.tile([HEAD_DIM + 1, 2, CHUNK], F32, tag="ctxu", bufs=2)
        nc.scalar.copy(ctxu[0:64, 0, 0:qw], ctx_e[0:64, :])
        nc.scalar.copy(ctxu[0:64, 1, 0:qw], ctx_o[0:64, :])
        bc_e = psum.tile([P, 2 * CHUNK], F32, tag="s_pair", bufs=2,
                         name="bc_e")[0:HEAD_DIM, 0:qw]
        bc_o = psum.tile([P, 2 * CHUNK], F32, tag="s_pair", bufs=2,
                         name="bc_o")[0:HEAD_DIM, 0:qw]
        for hl in range(2):
            mm(bc_e if hl == 0 else bc_o,
               ones64[64:65, :],
               recip_sb[64:65, hl, :],
               start=True, stop=True)
        nc.vector.tensor_mul(ctxT_sb[0:64, p, qlo:qlo + qw],
                             ctxu[0:64, 0, 0:qw], bc_e[:])
        tmp_o = npool.tile([HEAD_DIM, CHUNK], BF16, tag="tmp_o", bufs=2,
                           name="tmp_o")[:, 0:qw]
        nc.vector.tensor_mul(tmp_o[:], ctxu[0:64, 1, 0:qw], bc_o[:])
        return tmp_o
    def do_norm():
        # each half's reciprocal follows its copy immediately so the bc
        # broadcast matmuls (and with them the next oproj filler batch)
        # unblock ~1.5us earlier at a chunk boundary
        ctxu = npool.tile([HEAD_DIM + 1, 2, CHUNK], F32, tag="ctxu", bufs=2)
        with nc.allow_low_precision(reason="softmax denominators are O(1e3); fp32r's 11-bit mantissa is plenty"):
            nc.vector.tensor_copy(ctxu[:, 0, 0:qw], ctx_e[:])
            nc.vector.reciprocal(recip_sb[64:65, 0, :], ctxu[64:65, 0, 0:qw])
            nc.vector.tensor_copy(ctxu[:, 1, 0:qw], ctx_o[:])
            nc.vector.reciprocal(recip_sb[64:65, 1, :], ctxu[64:65, 1, 0:qw])
        if bc_pool is not None:
            # deferred norm: the bc tiles borrow the projection psum banks —
            # the ctx-tag rotation is already owned by the next chunk here
            bc_e = bc_pool.tile([HEAD_DIM, qw], F32, tag="ps_k", bufs=1,
                                name="bc_e")
            bc_o = bc_pool.tile([HEAD_DIM, qw], F32, tag="ps_q", bufs=1,
                                name="bc_o")
        else:
            bc_e = psum.tile([HEAD_DIM + 1, CHUNK], F32, tag="ctx_e",
                             bufs=1, name="bc_e")[0:HEAD_DIM, 0:qw]
            bc_o = psum.tile([HEAD_DIM + 1, CHUNK], F32, tag="ctx_o",
                             bufs=1, name="bc_o")[0:HEAD_DIM, 0:qw]
        for hl in range(2):
            mm(bc_e if hl == 0 else bc_o,
               ones64[64:65, :],
               recip_sb[64:65, hl, :],
               start=True, stop=True)
        nc.vector.tensor_mul(ctxT_sb[0:64, p, qlo:qlo + qw],
                             ctxu[0:64, 0, 0:qw], bc_e[:])
        tmp_o = npool.tile([HEAD_DIM, CHUNK], BF16, tag="tmp_o", bufs=2)
        nc.vector.tensor_mul(tmp_o[:, 0:qw], ctxu[0:64, 1, 0:qw], bc_o[:])
        nc.sync.dma_start(out=ctxT_sb[64:128, p, qlo:qlo + qw],
                          in_=tmp_o[:, 0:qw])

    if defer_norm:
        return do_norm
    do_norm()
    return None


def build_program(masked=False, with_bias=False):
    key = (masked, with_bias)
    if key in _PROGRAM_CACHE:
        return _PROGRAM_CACHE[key]
    nc = bacc.Bacc("TRN2", target_bir_lowering=False, debug=False,
                   enable_asserts=False)
    x_hi = nc.dram_tensor("x_hi", [H, S], E4M3, kind="ExternalInput").ap()
    x_lo = nc.dram_tensor("x_lo", [H, S], E4M3, kind="ExternalInput").ap()
    wq_hl = nc.dram_tensor("wq_hl", [H, 2, HD], E4M3, kind="ExternalInput").ap()
    wk_hl = nc.dram_tensor("wk_hl", [H, 2, HD], E4M3, kind="ExternalInput").ap()
    wv_hl = nc.dram_tensor("wv_hl", [H, 2, HD], E4M3, kind="ExternalInput").ap()
    wo = nc.dram_tensor("wo", [HD, H], BF16, kind="ExternalInput").ap()
    bq = nc.dram_tensor("bq", [1, HD], F32R, kind="ExternalInput").ap()
    bk = nc.dram_tensor("bk", [1, HD], F32R, kind="ExternalInput").ap()
    bv = nc.dram_tensor("bv", [1, HD], F32R, kind="ExternalInput").ap()
    am = nc.dram_tensor("am", [P, KT_S], F32, kind="ExternalInput").ap()
    o = nc.dram_tensor("o_part", [S, H], BF16, kind="ExternalOutput").ap()
    with tile.TileContext(nc) as tc:
        _emit(tc, nc, (x_hi, x_lo, wq_hl, wk_hl, wv_hl,
                       wo, bq, bk, bv, am, o, None), masked, with_bias)
    nc.compile()
    _PROGRAM_CACHE[key] = nc
    return nc


def _round_fp32r(a):
    """Round fp32 to the PE's fp32r format (11 mantissa bits, RNE)."""
    u = np.ascontiguousarray(a, np.float32).view(np.uint32)
    r = (u + np.uint32(0x7FF) + ((u >> np.uint32(12)) & np.uint32(1))) \
        & np.uint32(0xFFFFF000)
    return r.view(np.float32)


def make_in_maps(hidden_states, attention_mask, Wq, bq, Wk, bk, Wv, bv, Wo, bo):
    """Per-core input dicts. Core c: batch c//4, head-group c%4.

    Wq/bq are pre-scaled by 1/8 (= 1/sqrt(HEAD_DIM), exact in fp32) so the
    kernel's raw scores are already scaled. Tensors feeding float32r
    matmuls are pre-rounded to fp32r on the host (the device DMAs them
    into float32r tiles verbatim).
    """
    import ml_dtypes
    bf = ml_dtypes.bfloat16
    e4 = ml_dtypes.float8_e4m3

    def split_e4m3(a):
        hi = np.ascontiguousarray(a).astype(e4)
        lo = np.ascontiguousarray(a - hi.astype(np.float32)).astype(e4)
        return hi, lo

    hidden_states = np.asarray(hidden_states, np.float32)
    attention_mask = np.asarray(attention_mask, np.float32)
    xs = [split_e4m3(np.ascontiguousarray(hidden_states[b].T)) for b in range(B)]
    ams = []
    for b in range(B):
        amask = ((1.0 - attention_mask[b]) * -10000.0).astype(np.float32)
        ams.append(np.ascontiguousarray(amask.reshape(KT_S, P).T))
    in_maps = []
    for c in range(N_CORES):
        b, g = divmod(c, GROUPS)
        hs = slice(g * HD, (g + 1) * HD)
        # W scaled by 64 into e4m3's normal range; q*k picks up 64*64,
        # undone (with the 1/8 softmax scale) by the exp's 2^-15 scale;
        # v's 64 is undone by the 1/64 recip broadcast.
        wqh, wql = split_e4m3(np.asarray(Wq, np.float32)[hs, :].T * np.float32(64.0))
        wkh, wkl = split_e4m3(np.asarray(Wk, np.float32)[hs, :].T * np.float32(64.0))
        wvh, wvl = split_e4m3(np.asarray(Wv, np.float32)[hs, :].T * np.float32(64.0))
        in_maps.append({
            "x_hi": xs[b][0], "x_lo": xs[b][1],
            "wq_hl": np.ascontiguousarray(np.stack([wqh, wql], axis=1)),
            "wk_hl": np.ascontiguousarray(np.stack([wkh, wkl], axis=1)),
            "wv_hl": np.ascontiguousarray(np.stack([wvh, wvl], axis=1)),
            "wo": np.ascontiguousarray(np.asarray(Wo, np.float32)[:, hs].T).astype(bf),
            "bq": _round_fp32r(np.asarray(bq, np.float32)[hs].reshape(1, HD) * np.float32(64.0)),
            "bk": _round_fp32r(np.asarray(bk, np.float32)[hs].reshape(1, HD) * np.float32(64.0)),
            "bv": _round_fp32r(np.asarray(bv, np.float32)[hs].reshape(1, HD) * np.float32(64.0)),
            "am": ams[b],
        })
    return in_maps


def kernel(hidden_states, attention_mask, Wq, bq, Wk, bk, Wv, bv, Wo, bo):
    masked = not bool(np.all(np.asarray(attention_mask) == 1.0))
    with_bias = not (np.all(np.asarray(bq) == 0) and np.all(np.asarray(bk) == 0)
                     and np.all(np.asarray(bv) == 0))
    nc = build_program(masked, with_bias)
    in_maps = make_in_maps(hidden_states, attention_mask,
                           Wq, bq, Wk, bk, Wv, bv, Wo, bo)
    res = run_bass_kernel_spmd(nc, in_maps, core_ids=list(range(N_CORES)))
    out = np.zeros((B, S, H), np.float32)
    for c in range(N_CORES):
        b = c // GROUPS
        out[b] += np.asarray(res.results[c]["o_part"], np.float32)
    out += np.asarray(bo, np.float32)
    return out

